# revision 20
# baseline (speedup 1.0000x reference)
"""MoCo grouped-queue logits kernel for Trainium2 (8 NeuronCores, Bass/Tile).

Computation (reference):
    q = l2norm(im_q @ W_q)          # [N, C]
    k = l2norm(im_k @ W_k)          # [N, C]
    l_pos[n] = q[n] . k[n]
    route[n] = (label[n] - 1) % 4
    l_neg[n, :] = q[n] @ queues[route[n]]    # [N, K]
    logits = concat([l_pos, l_neg], 1) / T   # [N, 1+K]
    labels = zeros(N)

Strategy:
  - Queues are sharded along K across the 8 cores ([4, 128, K/8] each);
    each core computes all N samples against its K-slice. Each queue
    byte is read exactly once chip-wide.
  - Samples are SORTED by route group on the host, so l_neg becomes a
    few dense [cnt<=128, 512]-tile matmuls, one group per tile — no
    masking and no 4x redundant PE work. The tile plan depends only on
    the per-group histogram; compiled programs are cached per plan.
  - The q-projection (needed by every core) is replicated; the l_pos
    path (q.k) only needs per-sample values, so its projections are
    sharded: core i computes l_pos for sorted samples [64i, 64i+64).
  - Host work is layout only: transpose/tile inputs, sort rows,
    unsort output rows.
"""

import numpy as np

# Problem constants (hardcoded; kernel.py must be self-contained).
N = 512          # batch
D = 2048         # input feature dim
C = 128          # embedding dim
K = 65536        # queue length
G = 4            # number of queues
T = 0.07         # softmax temperature
NCORES = 8
KSH = K // NCORES            # 8192 queue columns per core
DT = D // 128                # 16 contraction tiles for the projections
NLP = N // NCORES            # 64 l_pos samples per core
CW = 2048                    # queue-chunk width (columns per DMA chunk)
NKC = KSH // CW              # 4 chunks per core
NSUB = CW // 512             # 4 matmuls (N=512) per chunk

_prog_cache = {}


def _plan_from_counts(counts):
    """M-tile plan: list of (row0, cnt<=128, group) over sorted rows.

    Groups with more than 128 rows are covered by full 128-row tiles whose
    last tile is [end-128, end) — tiles may OVERLAP (overlapped rows are
    computed and written twice with identical values). This keeps nearly
    every output DMA at full 128 partitions, which balances the SDMA
    engines (partial-partition DMAs concentrate on a few engines).
    """
    tiles = []
    r0 = 0
    for g in range(G):
        c = int(counts[g])
        if c == 0:
            continue
        if c <= 128:
            tiles.append((r0, c, g))
        else:
            off = 0
            while off + 128 < c:
                tiles.append((r0 + off, 128, g))
                off += 128
            tiles.append((r0 + c - 128, 128, g))
        r0 += c
    return tuple(tiles)


def _build(plan):
    """Build + compile the Bass program for one tile plan."""
    import concourse.tile as tile
    from concourse import bacc, mybir

    f32 = mybir.dt.float32
    AF = mybir.ActivationFunctionType

    nc = bacc.Bacc("TRN2", target_bir_lowering=False, debug=False,
                   num_devices=NCORES)

    # Inputs, pre-tiled on host so every DMA is partition-contiguous.
    #   imqt  [128, DT*512]: [p, t*512+n] = im_q_sorted[n, t*128+p]
    #   wqt   [128, DT*128]: [p, t*128+c] = W_q[t*128+p, c]  (same wkt)
    #   imqlp/imklp [128, DT*64]: this core's 64 sorted samples
    #   qsh   [G, 128, KSH]: this core's K-slice of the queues
    imqt = nc.dram_tensor("imqt", [128, DT * 512], f32, kind="ExternalInput")
    wqt = nc.dram_tensor("wqt", [128, DT * 128], f32, kind="ExternalInput")
    wkt = nc.dram_tensor("wkt", [128, DT * 128], f32, kind="ExternalInput")
    imqlp = nc.dram_tensor("imqlp", [128, DT * NLP], f32, kind="ExternalInput")
    imklp = nc.dram_tensor("imklp", [128, DT * NLP], f32, kind="ExternalInput")
    qsh = nc.dram_tensor("qsh", [G, 128, KSH], f32, kind="ExternalInput")
    # Outputs (sorted row order): lneg [N, KSH], lpos [1, NLP] (scaled 1/T).
    lneg = nc.dram_tensor("lneg", [N, KSH], f32, kind="ExternalOutput")
    lpos = nc.dram_tensor("lpos", [1, NLP], f32, kind="ExternalOutput")

    used_groups = sorted({g for _, _, g in plan})
    ntiles = len(plan)

    with tile.TileContext(nc) as tc:
        with tc.tile_pool(name="pers", bufs=1) as pers:
            ones_col = pers.tile([128, 1], f32)
            nc.vector.memset(ones_col[:], 1.0)
            eps_bias = pers.tile([1, 1], f32)
            nc.vector.memset(eps_bias[:], 1e-24)
            eps_biasp = pers.tile([128, 1], f32)
            nc.vector.memset(eps_biasp[:], 1e-24)
            # Sorted UNNORMALIZED qT: the stationary operand. The l2-norm
            # scaling (and 1/T) is applied later, fused into the
            # PSUM->SBUF drain of each output tile — this keeps the norm
            # chain off the critical path to the first l_neg matmul.
            qtu = pers.tile([128, N], f32)
            # Per-tile 1/||q|| vectors, [cnt, 1] each, in columns of one tile.
            inv_sb = pers.tile([128, ntiles], f32)
            inv_t_sb = pers.tile([128, ntiles], f32)

            # ---- Phase A: projections, norms, l_pos shard ----
            # Ring assignment: ALL loads (inputs, then queue chunks) ride
            # the SP ring (nc.sync) so chunk prefetch never queues behind
            # compute; outputs ride the ACT ring (nc.scalar), where each
            # out-DMA issues right after its drain in the same FIFO.
            with tc.tile_pool(name="pa", bufs=1) as pa, \
                 tc.tile_pool(name="paps", bufs=1, space="PSUM") as paps:
                wq_sb = pa.tile([128, DT * 128], f32)
                nc.sync.dma_start(wq_sb[:], wqt[:])
                # imqt in quarters so projection matmuls overlap the load.
                imq_sb = pa.tile([128, DT * 512], f32)
                QT = DT // 4
                for qtr in range(4):
                    nc.sync.dma_start(
                        imq_sb[:, qtr * QT * 512:(qtr + 1) * QT * 512],
                        imqt[:, qtr * QT * 512:(qtr + 1) * QT * 512])
                imqlp_sb = pa.tile([128, DT * NLP], f32)
                nc.sync.dma_start(imqlp_sb[:], imqlp[:])
                imklp_sb = pa.tile([128, DT * NLP], f32)
                nc.sync.dma_start(imklp_sb[:], imklp[:])
                wk_sb = pa.tile([128, DT * 128], f32)
                nc.sync.dma_start(wk_sb[:], wkt[:])

                # qT[c, n] = sum_d W_q[d, c] im_q[n, d]  (all N samples)
                qt_ps = paps.tile([128, N], f32)
                for t in range(DT):
                    nc.tensor.matmul(qt_ps[:],
                                     wq_sb[:, t * 128:(t + 1) * 128],
                                     imq_sb[:, t * 512:(t + 1) * 512],
                                     start=(t == 0), stop=(t == DT - 1))
                nc.vector.tensor_copy(qtu[:], qt_ps[:])

                # l_pos shard projections (64 samples each for q and k),
                # sharing one PSUM bank in disjoint column ranges.
                aux_ps = paps.tile([128, 512], f32)
                lp_ps = aux_ps[:, 0:2 * NLP]
                for t in range(DT):
                    nc.tensor.matmul(lp_ps[:, 0:NLP],
                                     wq_sb[:, t * 128:(t + 1) * 128],
                                     imqlp_sb[:, t * NLP:(t + 1) * NLP],
                                     start=(t == 0), stop=(t == DT - 1))
                for t in range(DT):
                    nc.tensor.matmul(lp_ps[:, NLP:2 * NLP],
                                     wk_sb[:, t * 128:(t + 1) * 128],
                                     imklp_sb[:, t * NLP:(t + 1) * NLP],
                                     start=(t == 0), stop=(t == DT - 1))

                # Per-tile inverse norms: ssq_tile = (qT^2)[:, r0:r0+cnt]
                # summed over partitions via an N=1 matmul, then
                # 1/sqrt(ssq+eps) per partition. Off the critical path.
                sq_sb = pa.tile([128, N], f32)
                nc.vector.tensor_mul(sq_sb[:], qtu[:], qtu[:])
                misc_ps = aux_ps[:, 2 * NLP:2 * NLP + 64]
                inv1 = pa.tile([128, ntiles], f32)
                for ti, (r0, cnt, g) in enumerate(plan):
                    nc.tensor.matmul(misc_ps[0:cnt, ti:ti + 1],
                                     sq_sb[:, r0:r0 + cnt], ones_col[:],
                                     start=True, stop=True)
                    nc.scalar.activation(inv1[0:cnt, ti:ti + 1],
                                         misc_ps[0:cnt, ti:ti + 1],
                                         AF.Sqrt, bias=eps_biasp[0:cnt, :])
                    nc.vector.reciprocal(inv_sb[0:cnt, ti:ti + 1],
                                         inv1[0:cnt, ti:ti + 1])
                    nc.vector.tensor_scalar_mul(
                        inv_t_sb[0:cnt, ti:ti + 1],
                        inv_sb[0:cnt, ti:ti + 1], 1.0 / T)

                # l_pos shard: dot * invq_lp * invk_lp / T.
                qlp_sb = pa.tile([128, NLP], f32)
                nc.vector.tensor_copy(qlp_sb[:], lp_ps[:, 0:NLP])
                klp_sb = pa.tile([128, NLP], f32)
                nc.vector.tensor_copy(klp_sb[:], lp_ps[:, NLP:2 * NLP])
                red_sb = pa.tile([128, 3 * NLP], f32)
                nc.vector.tensor_mul(red_sb[:, 0:NLP],
                                     qlp_sb[:], qlp_sb[:])
                nc.vector.tensor_mul(red_sb[:, NLP:2 * NLP],
                                     klp_sb[:], klp_sb[:])
                nc.vector.tensor_mul(red_sb[:, 2 * NLP:3 * NLP],
                                     qlp_sb[:], klp_sb[:])
                red_ps = aux_ps[0:1, 2 * NLP + 64:5 * NLP + 64]
                nc.tensor.matmul(red_ps[:], ones_col[:], red_sb[:],
                                 start=True, stop=True)
                norml = pa.tile([1, 2 * NLP], f32)
                nc.scalar.activation(norml[:], red_ps[:, 0:2 * NLP], AF.Sqrt,
                                     bias=eps_bias[:])
                invl = pa.tile([1, 2 * NLP], f32)
                nc.vector.reciprocal(invl[:], norml[:])
                lp1 = pa.tile([1, NLP], f32)
                nc.vector.tensor_mul(lp1[:], invl[:, 0:NLP], invl[:, NLP:2 * NLP])
                lp2 = pa.tile([1, NLP], f32)
                nc.vector.tensor_mul(lp2[:], lp1[:], red_ps[:, 2 * NLP:3 * NLP])
                lp3 = pa.tile([1, NLP], f32)
                nc.vector.tensor_scalar_mul(lp3[:], lp2[:], 1.0 / T)
                nc.scalar.dma_start(lpos[:], lp3[:])

            # ---- Phase C: l_neg tiles over the queue shard ----
            # fresh0[ti]: first row of tile ti not already written by the
            # previous (overlapping) tile — only fresh rows go to HBM.
            fresh0 = []
            prev_end = 0
            for r0, cnt, g in plan:
                fresh0.append(max(r0, prev_end))
                prev_end = max(prev_end, r0 + cnt)

            with tc.tile_pool(name="qp", bufs=3) as qp, \
                 tc.tile_pool(name="sp", bufs=3) as sp, \
                 tc.tile_pool(name="cps", bufs=3, space="PSUM") as cps:
                for kc in range(NKC):
                    qch = {}
                    for g in used_groups:
                        qt_ = qp.tile([128, CW], f32, tag=f"qch{g}",
                                      name=f"qch{g}_{kc}")
                        nc.sync.dma_start(
                            qt_[:], qsh[g, :, kc * CW:(kc + 1) * CW])
                        qch[g] = qt_
                    for ti, (r0, cnt, g) in enumerate(plan):
                        stg = sp.tile([128, CW], f32, tag="stg",
                                      name=f"stg_{kc}_{ti}")
                        # Two-bank PSUM tiles; each drained by a fused
                        # scale-copy, alternating DVE/ACT so the drain
                        # engine never gates the PE.
                        for half in range(CW // 1024):
                            hw = 1024
                            ps = cps.tile([128, hw], f32, tag="ps",
                                          name=f"ps_{kc}_{ti}_{half}")
                            for sub in range(hw // 512):
                                c0 = half * hw + sub * 512
                                nc.tensor.matmul(
                                    ps[:cnt, sub * 512:(sub + 1) * 512],
                                    qtu[:, r0:r0 + cnt],
                                    qch[g][:, c0:c0 + 512],
                                    start=True, stop=True)
                            if half % 2 == 0:
                                nc.vector.tensor_scalar(
                                    stg[:cnt, half * hw:(half + 1) * hw],
                                    ps[:cnt, :],
                                    inv_sb[0:cnt, ti:ti + 1], 1.0 / T,
                                    op0=mybir.AluOpType.mult,
                                    op1=mybir.AluOpType.mult)
                            else:
                                nc.scalar.activation(
                                    stg[:cnt, half * hw:(half + 1) * hw],
                                    ps[:cnt, :], AF.Copy,
                                    scale=inv_t_sb[0:cnt, ti:ti + 1])
                        f0 = fresh0[ti] - r0
                        nc.scalar.dma_start(
                            lneg[fresh0[ti]:r0 + cnt, kc * CW:(kc + 1) * CW],
                            stg[f0:cnt, :])

    nc.compile()
    return nc


def _get_program(plan):
    if plan not in _prog_cache:
        _prog_cache[plan] = _build(plan)
    return _prog_cache[plan]


def _tile_cols(x, ncols):
    """[n, D] -> [128, DT*n] with [p, t*n+j] = x[j, t*128+p]."""
    n = x.shape[0]
    assert n == ncols
    return np.ascontiguousarray(
        x.T.reshape(DT, 128, n).transpose(1, 0, 2).reshape(128, DT * n))


def _stage_inputs(im_q, im_k, W_q, W_k, queues, label):
    f32 = np.float32
    im_q = np.asarray(im_q, dtype=f32)
    im_k = np.asarray(im_k, dtype=f32)
    W_q = np.asarray(W_q, dtype=f32)
    W_k = np.asarray(W_k, dtype=f32)
    queues = np.asarray(queues, dtype=f32)
    label = np.asarray(label)

    route = ((label.astype(np.int64) - 1) % G).astype(np.int64)
    order = np.argsort(route, kind="stable")
    counts = np.bincount(route, minlength=G)
    plan = _plan_from_counts(counts)

    im_q_s = im_q[order]
    im_k_s = im_k[order]

    imqt = _tile_cols(im_q_s, N)
    wqt = np.ascontiguousarray(
        W_q.reshape(DT, 128, C).transpose(1, 0, 2).reshape(128, DT * C))
    wkt = np.ascontiguousarray(
        W_k.reshape(DT, 128, C).transpose(1, 0, 2).reshape(128, DT * C))

    in_maps = []
    for i in range(NCORES):
        sl = slice(i * NLP, (i + 1) * NLP)
        in_maps.append({
            "imqt": imqt, "wqt": wqt, "wkt": wkt,
            "imqlp": _tile_cols(im_q_s[sl], NLP),
            "imklp": _tile_cols(im_k_s[sl], NLP),
            "qsh": np.ascontiguousarray(queues[:, :, i * KSH:(i + 1) * KSH]),
        })
    return plan, order, in_maps


def kernel(im_q, im_k, W_q, W_k, queues, label):
    from concourse.bass_utils import run_bass_kernel_spmd

    plan, order, in_maps = _stage_inputs(im_q, im_k, W_q, W_k, queues, label)
    nc = _get_program(plan)
    res = run_bass_kernel_spmd(nc, in_maps, core_ids=list(range(NCORES)))

    logits = np.empty((N, 1 + K), dtype=np.float32)
    lpos_sorted = np.concatenate(
        [res.results[i]["lpos"][0] for i in range(NCORES)])
    logits[order, 0] = lpos_sorted
    for i in range(NCORES):
        logits[order, 1 + i * KSH:1 + (i + 1) * KSH] = res.results[i]["lneg"]
    labels = np.zeros(N, dtype=np.int32)
    return logits, labels


# revision 21
# speedup vs baseline: 1.0144x; 1.0144x over previous
"""MoCo grouped-queue logits kernel for Trainium2 (8 NeuronCores, Bass/Tile).

Computation (reference):
    q = l2norm(im_q @ W_q)          # [N, C]
    k = l2norm(im_k @ W_k)          # [N, C]
    l_pos[n] = q[n] . k[n]
    route[n] = (label[n] - 1) % 4
    l_neg[n, :] = q[n] @ queues[route[n]]    # [N, K]
    logits = concat([l_pos, l_neg], 1) / T   # [N, 1+K]
    labels = zeros(N)

Strategy:
  - Queues are sharded along K across the 8 cores ([4, 128, K/8] each);
    each core computes all N samples against its K-slice. Each queue
    byte is read exactly once chip-wide.
  - Samples are SORTED by route group on the host, so l_neg becomes a
    few dense [cnt<=128, 512]-tile matmuls, one group per tile — no
    masking and no 4x redundant PE work. The tile plan depends only on
    the per-group histogram; compiled programs are cached per plan.
  - The q-projection (needed by every core) is replicated; the l_pos
    path (q.k) only needs per-sample values, so its projections are
    sharded: core i computes l_pos for sorted samples [64i, 64i+64).
  - Host work is layout only: transpose/tile inputs, sort rows,
    unsort output rows.
"""

import numpy as np

# Problem constants (hardcoded; kernel.py must be self-contained).
N = 512          # batch
D = 2048         # input feature dim
C = 128          # embedding dim
K = 65536        # queue length
G = 4            # number of queues
T = 0.07         # softmax temperature
NCORES = 8
KSH = K // NCORES            # 8192 queue columns per core
DT = D // 128                # 16 contraction tiles for the projections
NLP = N // NCORES            # 64 l_pos samples per core
CW = 2048                    # queue-chunk width (columns per DMA chunk)
NKC = KSH // CW              # 4 chunks per core
NSUB = CW // 512             # 4 matmuls (N=512) per chunk

_prog_cache = {}


def _plan_from_counts(counts):
    """M-tile plan: list of (row0, cnt<=128, group) over sorted rows.

    Groups with more than 128 rows are covered by full 128-row tiles whose
    last tile is [end-128, end) — tiles may OVERLAP (overlapped rows are
    computed and written twice with identical values). This keeps nearly
    every output DMA at full 128 partitions, which balances the SDMA
    engines (partial-partition DMAs concentrate on a few engines).
    """
    tiles = []
    r0 = 0
    for g in range(G):
        c = int(counts[g])
        if c == 0:
            continue
        if c <= 128:
            tiles.append((r0, c, g))
        else:
            off = 0
            while off + 128 < c:
                tiles.append((r0 + off, 128, g))
                off += 128
            tiles.append((r0 + c - 128, 128, g))
        r0 += c
    return tuple(tiles)


def _build(plan):
    """Build + compile the Bass program for one tile plan."""
    import concourse.tile as tile
    from concourse import bacc, mybir

    f32 = mybir.dt.float32
    AF = mybir.ActivationFunctionType

    nc = bacc.Bacc("TRN2", target_bir_lowering=False, debug=False,
                   num_devices=NCORES)

    # Inputs, pre-tiled on host so every DMA is partition-contiguous.
    #   imqt  [128, DT*512]: [p, t*512+n] = im_q_sorted[n, t*128+p]
    #   wqt   [128, DT*128]: [p, t*128+c] = W_q[t*128+p, c]  (same wkt)
    #   imqlp/imklp [128, DT*64]: this core's 64 sorted samples
    #   qsh   [G, 128, KSH]: this core's K-slice of the queues
    imqt = nc.dram_tensor("imqt", [128, DT * 512], f32, kind="ExternalInput")
    wqt = nc.dram_tensor("wqt", [128, DT * 128], f32, kind="ExternalInput")
    wkt = nc.dram_tensor("wkt", [128, DT * 128], f32, kind="ExternalInput")
    imqlp = nc.dram_tensor("imqlp", [128, DT * NLP], f32, kind="ExternalInput")
    imklp = nc.dram_tensor("imklp", [128, DT * NLP], f32, kind="ExternalInput")
    qsh = nc.dram_tensor("qsh", [G, 128, KSH], f32, kind="ExternalInput")
    # Outputs (sorted row order): lneg [N, KSH], lpos [1, NLP] (scaled 1/T).
    lneg = nc.dram_tensor("lneg", [N, KSH], f32, kind="ExternalOutput")
    lpos = nc.dram_tensor("lpos", [1, NLP], f32, kind="ExternalOutput")

    used_groups = sorted({g for _, _, g in plan})
    ntiles = len(plan)

    with tile.TileContext(nc) as tc:
        with tc.tile_pool(name="pers", bufs=1) as pers:
            ones_col = pers.tile([128, 1], f32)
            nc.vector.memset(ones_col[:], 1.0)
            eps_bias = pers.tile([1, 1], f32)
            nc.vector.memset(eps_bias[:], 1e-24)
            eps_biasp = pers.tile([128, 1], f32)
            nc.vector.memset(eps_biasp[:], 1e-24)
            # Sorted UNNORMALIZED qT: the stationary operand. The l2-norm
            # scaling (and 1/T) is applied later, fused into the
            # PSUM->SBUF drain of each output tile — this keeps the norm
            # chain off the critical path to the first l_neg matmul.
            qtu = pers.tile([128, N], f32)
            # Per-tile 1/||q|| vectors, [cnt, 1] each, in columns of one tile.
            inv_sb = pers.tile([128, ntiles], f32)
            inv_t_sb = pers.tile([128, ntiles], f32)

            # ---- Phase A: projections, norms, l_pos shard ----
            # Ring assignment: ALL loads (inputs, then queue chunks) ride
            # the SP ring (nc.sync) so chunk prefetch never queues behind
            # compute; outputs ride the ACT ring (nc.scalar), where each
            # out-DMA issues right after its drain in the same FIFO.
            with tc.tile_pool(name="pa", bufs=1) as pa, \
                 tc.tile_pool(name="paps", bufs=1, space="PSUM") as paps:
                wq_sb = pa.tile([128, DT * 128], f32)
                nc.sync.dma_start(wq_sb[:], wqt[:])
                # imqt in quarters so projection matmuls overlap the load.
                imq_sb = pa.tile([128, DT * 512], f32)
                QT = DT // 4
                for qtr in range(4):
                    nc.sync.dma_start(
                        imq_sb[:, qtr * QT * 512:(qtr + 1) * QT * 512],
                        imqt[:, qtr * QT * 512:(qtr + 1) * QT * 512])
                imqlp_sb = pa.tile([128, DT * NLP], f32)
                nc.sync.dma_start(imqlp_sb[:], imqlp[:])
                imklp_sb = pa.tile([128, DT * NLP], f32)
                nc.sync.dma_start(imklp_sb[:], imklp[:])
                wk_sb = pa.tile([128, DT * 128], f32)
                nc.sync.dma_start(wk_sb[:], wkt[:])

                # qT[c, n] = sum_d W_q[d, c] im_q[n, d]  (all N samples)
                qt_ps = paps.tile([128, N], f32)
                for t in range(DT):
                    nc.tensor.matmul(qt_ps[:],
                                     wq_sb[:, t * 128:(t + 1) * 128],
                                     imq_sb[:, t * 512:(t + 1) * 512],
                                     start=(t == 0), stop=(t == DT - 1))
                nc.vector.tensor_copy(qtu[:], qt_ps[:])

                # l_pos shard projections (64 samples each for q and k),
                # sharing one PSUM bank in disjoint column ranges.
                aux_ps = paps.tile([128, 512], f32)
                lp_ps = aux_ps[:, 0:2 * NLP]
                for t in range(DT):
                    nc.tensor.matmul(lp_ps[:, 0:NLP],
                                     wq_sb[:, t * 128:(t + 1) * 128],
                                     imqlp_sb[:, t * NLP:(t + 1) * NLP],
                                     start=(t == 0), stop=(t == DT - 1))
                for t in range(DT):
                    nc.tensor.matmul(lp_ps[:, NLP:2 * NLP],
                                     wk_sb[:, t * 128:(t + 1) * 128],
                                     imklp_sb[:, t * NLP:(t + 1) * NLP],
                                     start=(t == 0), stop=(t == DT - 1))

                # Per-tile inverse norms: ssq_tile = (qT^2)[:, r0:r0+cnt]
                # summed over partitions via an N=1 matmul, then
                # 1/sqrt(ssq+eps) per partition. Off the critical path.
                sq_sb = pa.tile([128, N], f32)
                nc.vector.tensor_mul(sq_sb[:], qtu[:], qtu[:])
                misc_ps = aux_ps[:, 2 * NLP:2 * NLP + 64]
                inv1 = pa.tile([128, ntiles], f32)
                for ti, (r0, cnt, g) in enumerate(plan):
                    nc.tensor.matmul(misc_ps[0:cnt, ti:ti + 1],
                                     sq_sb[:, r0:r0 + cnt], ones_col[:],
                                     start=True, stop=True)
                    nc.scalar.activation(inv1[0:cnt, ti:ti + 1],
                                         misc_ps[0:cnt, ti:ti + 1],
                                         AF.Sqrt, bias=eps_biasp[0:cnt, :])
                    nc.vector.reciprocal(inv_sb[0:cnt, ti:ti + 1],
                                         inv1[0:cnt, ti:ti + 1])
                    nc.vector.tensor_scalar_mul(
                        inv_t_sb[0:cnt, ti:ti + 1],
                        inv_sb[0:cnt, ti:ti + 1], 1.0 / T)

                # l_pos shard: dot * invq_lp * invk_lp / T.
                qlp_sb = pa.tile([128, NLP], f32)
                nc.vector.tensor_copy(qlp_sb[:], lp_ps[:, 0:NLP])
                klp_sb = pa.tile([128, NLP], f32)
                nc.vector.tensor_copy(klp_sb[:], lp_ps[:, NLP:2 * NLP])
                red_sb = pa.tile([128, 3 * NLP], f32)
                nc.vector.tensor_mul(red_sb[:, 0:NLP],
                                     qlp_sb[:], qlp_sb[:])
                nc.vector.tensor_mul(red_sb[:, NLP:2 * NLP],
                                     klp_sb[:], klp_sb[:])
                nc.vector.tensor_mul(red_sb[:, 2 * NLP:3 * NLP],
                                     qlp_sb[:], klp_sb[:])
                red_ps = aux_ps[0:1, 2 * NLP + 64:5 * NLP + 64]
                nc.tensor.matmul(red_ps[:], ones_col[:], red_sb[:],
                                 start=True, stop=True)
                norml = pa.tile([1, 2 * NLP], f32)
                nc.scalar.activation(norml[:], red_ps[:, 0:2 * NLP], AF.Sqrt,
                                     bias=eps_bias[:])
                invl = pa.tile([1, 2 * NLP], f32)
                nc.vector.reciprocal(invl[:], norml[:])
                lp1 = pa.tile([1, NLP], f32)
                nc.vector.tensor_mul(lp1[:], invl[:, 0:NLP], invl[:, NLP:2 * NLP])
                lp2 = pa.tile([1, NLP], f32)
                nc.vector.tensor_mul(lp2[:], lp1[:], red_ps[:, 2 * NLP:3 * NLP])
                lp3 = pa.tile([1, NLP], f32)
                nc.vector.tensor_scalar_mul(lp3[:], lp2[:], 1.0 / T)
                nc.scalar.dma_start(lpos[:], lp3[:])

            # ---- Phase C: l_neg tiles over the queue shard ----
            # fresh0[ti]: first row of tile ti not already written by the
            # previous (overlapping) tile — only fresh rows go to HBM.
            fresh0 = []
            prev_end = 0
            for r0, cnt, g in plan:
                fresh0.append(max(r0, prev_end))
                prev_end = max(prev_end, r0 + cnt)

            with tc.tile_pool(name="qp", bufs=3) as qp, \
                 tc.tile_pool(name="sp", bufs=8) as sp, \
                 tc.tile_pool(name="cps", bufs=3, space="PSUM") as cps:
                for kc in range(NKC):
                    qch = {}
                    for g in used_groups:
                        qt_ = qp.tile([128, CW], f32, tag=f"qch{g}",
                                      name=f"qch{g}_{kc}")
                        nc.sync.dma_start(
                            qt_[:], qsh[g, :, kc * CW:(kc + 1) * CW])
                        qch[g] = qt_
                    for ti, (r0, cnt, g) in enumerate(plan):
                        f0 = fresh0[ti] - r0
                        # Two-bank PSUM tiles; each drained by a fused
                        # scale-copy (alternating DVE/ACT) into its own
                        # half-width stage slot, which goes straight out
                        # to HBM — small slots recycle fast, so out-DMA
                        # completion latency never stalls the drains.
                        for half in range(CW // 1024):
                            hw = 1024
                            ps = cps.tile([128, hw], f32, tag="ps",
                                          name=f"ps_{kc}_{ti}_{half}")
                            for sub in range(hw // 512):
                                c0 = half * hw + sub * 512
                                nc.tensor.matmul(
                                    ps[:cnt, sub * 512:(sub + 1) * 512],
                                    qtu[:, r0:r0 + cnt],
                                    qch[g][:, c0:c0 + 512],
                                    start=True, stop=True)
                            stg = sp.tile([128, hw], f32, tag="stg",
                                          name=f"stg_{kc}_{ti}_{half}")
                            if half % 2 == 0:
                                nc.vector.tensor_scalar(
                                    stg[:cnt, :], ps[:cnt, :],
                                    inv_sb[0:cnt, ti:ti + 1], 1.0 / T,
                                    op0=mybir.AluOpType.mult,
                                    op1=mybir.AluOpType.mult)
                            else:
                                nc.scalar.activation(
                                    stg[:cnt, :], ps[:cnt, :], AF.Copy,
                                    scale=inv_t_sb[0:cnt, ti:ti + 1])
                            nc.scalar.dma_start(
                                lneg[fresh0[ti]:r0 + cnt,
                                     kc * CW + half * hw:
                                     kc * CW + (half + 1) * hw],
                                stg[f0:cnt, :])

    nc.compile()
    return nc


def _get_program(plan):
    if plan not in _prog_cache:
        _prog_cache[plan] = _build(plan)
    return _prog_cache[plan]


def _tile_cols(x, ncols):
    """[n, D] -> [128, DT*n] with [p, t*n+j] = x[j, t*128+p]."""
    n = x.shape[0]
    assert n == ncols
    return np.ascontiguousarray(
        x.T.reshape(DT, 128, n).transpose(1, 0, 2).reshape(128, DT * n))


def _stage_inputs(im_q, im_k, W_q, W_k, queues, label):
    f32 = np.float32
    im_q = np.asarray(im_q, dtype=f32)
    im_k = np.asarray(im_k, dtype=f32)
    W_q = np.asarray(W_q, dtype=f32)
    W_k = np.asarray(W_k, dtype=f32)
    queues = np.asarray(queues, dtype=f32)
    label = np.asarray(label)

    route = ((label.astype(np.int64) - 1) % G).astype(np.int64)
    order = np.argsort(route, kind="stable")
    counts = np.bincount(route, minlength=G)
    plan = _plan_from_counts(counts)

    im_q_s = im_q[order]
    im_k_s = im_k[order]

    imqt = _tile_cols(im_q_s, N)
    wqt = np.ascontiguousarray(
        W_q.reshape(DT, 128, C).transpose(1, 0, 2).reshape(128, DT * C))
    wkt = np.ascontiguousarray(
        W_k.reshape(DT, 128, C).transpose(1, 0, 2).reshape(128, DT * C))

    in_maps = []
    for i in range(NCORES):
        sl = slice(i * NLP, (i + 1) * NLP)
        in_maps.append({
            "imqt": imqt, "wqt": wqt, "wkt": wkt,
            "imqlp": _tile_cols(im_q_s[sl], NLP),
            "imklp": _tile_cols(im_k_s[sl], NLP),
            "qsh": np.ascontiguousarray(queues[:, :, i * KSH:(i + 1) * KSH]),
        })
    return plan, order, in_maps


def kernel(im_q, im_k, W_q, W_k, queues, label):
    from concourse.bass_utils import run_bass_kernel_spmd

    plan, order, in_maps = _stage_inputs(im_q, im_k, W_q, W_k, queues, label)
    nc = _get_program(plan)
    res = run_bass_kernel_spmd(nc, in_maps, core_ids=list(range(NCORES)))

    logits = np.empty((N, 1 + K), dtype=np.float32)
    lpos_sorted = np.concatenate(
        [res.results[i]["lpos"][0] for i in range(NCORES)])
    logits[order, 0] = lpos_sorted
    for i in range(NCORES):
        logits[order, 1 + i * KSH:1 + (i + 1) * KSH] = res.results[i]["lneg"]
    labels = np.zeros(N, dtype=np.int32)
    return logits, labels


# revision 22
# speedup vs baseline: 1.0306x; 1.0160x over previous
"""MoCo grouped-queue logits kernel for Trainium2 (8 NeuronCores, Bass/Tile).

Computation (reference):
    q = l2norm(im_q @ W_q)          # [N, C]
    k = l2norm(im_k @ W_k)          # [N, C]
    l_pos[n] = q[n] . k[n]
    route[n] = (label[n] - 1) % 4
    l_neg[n, :] = q[n] @ queues[route[n]]    # [N, K]
    logits = concat([l_pos, l_neg], 1) / T   # [N, 1+K]
    labels = zeros(N)

Strategy:
  - Queues are sharded along K across the 8 cores ([4, 128, K/8] each);
    each core computes all N samples against its K-slice. Each queue
    byte is read exactly once chip-wide.
  - Samples are SORTED by route group on the host, so l_neg becomes a
    few dense [cnt<=128, 512]-tile matmuls, one group per tile — no
    masking and no 4x redundant PE work. The tile plan depends only on
    the per-group histogram; compiled programs are cached per plan.
  - The q-projection (needed by every core) is replicated; the l_pos
    path (q.k) only needs per-sample values, so its projections are
    sharded: core i computes l_pos for sorted samples [64i, 64i+64).
  - Host work is layout only: transpose/tile inputs, sort rows,
    unsort output rows.
"""

import numpy as np

# Problem constants (hardcoded; kernel.py must be self-contained).
N = 512          # batch
D = 2048         # input feature dim
C = 128          # embedding dim
K = 65536        # queue length
G = 4            # number of queues
T = 0.07         # softmax temperature
NCORES = 8
KSH = K // NCORES            # 8192 queue columns per core
DT = D // 128                # 16 contraction tiles for the projections
NLP = N // NCORES            # 64 l_pos samples per core
CW = 2048                    # queue-chunk width (columns per DMA chunk)
NKC = KSH // CW              # 4 chunks per core
NSUB = CW // 512             # 4 matmuls (N=512) per chunk

_prog_cache = {}


def _plan_from_counts(counts):
    """M-tile plan: list of (row0, cnt<=128, group) over sorted rows.

    Groups with more than 128 rows are covered by full 128-row tiles whose
    last tile is [end-128, end) — tiles may OVERLAP (overlapped rows are
    computed and written twice with identical values). This keeps nearly
    every output DMA at full 128 partitions, which balances the SDMA
    engines (partial-partition DMAs concentrate on a few engines).
    """
    tiles = []
    r0 = 0
    for g in range(G):
        c = int(counts[g])
        if c == 0:
            continue
        if c <= 128:
            tiles.append((r0, c, g))
        else:
            off = 0
            while off + 128 < c:
                tiles.append((r0 + off, 128, g))
                off += 128
            tiles.append((r0 + c - 128, 128, g))
        r0 += c
    return tuple(tiles)


def _build(plan):
    """Build + compile the Bass program for one tile plan."""
    import concourse.tile as tile
    from concourse import bacc, mybir

    f32 = mybir.dt.float32
    AF = mybir.ActivationFunctionType

    nc = bacc.Bacc("TRN2", target_bir_lowering=False, debug=False,
                   num_devices=NCORES)

    # Inputs, pre-tiled on host so every DMA is partition-contiguous.
    #   imqt  [128, DT*512]: [p, t*512+n] = im_q_sorted[n, t*128+p]
    #   wqt   [128, DT*128]: [p, t*128+c] = W_q[t*128+p, c]  (same wkt)
    #   imqlp/imklp [128, DT*64]: this core's 64 sorted samples
    #   qsh   [G, 128, KSH]: this core's K-slice of the queues
    imqt = nc.dram_tensor("imqt", [128, DT * 512], f32, kind="ExternalInput")
    wqt = nc.dram_tensor("wqt", [128, DT * 128], f32, kind="ExternalInput")
    wkt = nc.dram_tensor("wkt", [128, DT * 128], f32, kind="ExternalInput")
    imqlp = nc.dram_tensor("imqlp", [128, DT * NLP], f32, kind="ExternalInput")
    imklp = nc.dram_tensor("imklp", [128, DT * NLP], f32, kind="ExternalInput")
    qsh = nc.dram_tensor("qsh", [G, 128, KSH], f32, kind="ExternalInput")
    # Outputs (sorted row order): lneg [N, KSH], lpos [1, NLP] (scaled 1/T).
    lneg = nc.dram_tensor("lneg", [N, KSH], f32, kind="ExternalOutput")
    lpos = nc.dram_tensor("lpos", [1, NLP], f32, kind="ExternalOutput")

    used_groups = sorted({g for _, _, g in plan})
    ntiles = len(plan)

    with tile.TileContext(nc) as tc:
        with tc.tile_pool(name="pers", bufs=1) as pers:
            ones_col = pers.tile([128, 1], f32)
            nc.vector.memset(ones_col[:], 1.0)
            eps_bias = pers.tile([1, 1], f32)
            nc.vector.memset(eps_bias[:], 1e-24)
            eps_biasp = pers.tile([128, 1], f32)
            nc.vector.memset(eps_biasp[:], 1e-24)
            # Sorted UNNORMALIZED qT: the stationary operand. The l2-norm
            # scaling (and 1/T) is applied later, fused into the
            # PSUM->SBUF drain of each output tile — this keeps the norm
            # chain off the critical path to the first l_neg matmul.
            qtu = pers.tile([128, N], f32)
            # Per-tile 1/||q|| vectors, [cnt, 1] each, in columns of one tile.
            inv_sb = pers.tile([128, ntiles], f32)
            inv_t_sb = pers.tile([128, ntiles], f32)

            # ---- Phase A: projections, norms, l_pos shard ----
            # Ring assignment: ALL loads (inputs, then queue chunks) ride
            # the SP ring (nc.sync) so chunk prefetch never queues behind
            # compute; outputs ride the ACT ring (nc.scalar), where each
            # out-DMA issues right after its drain in the same FIFO.
            with tc.tile_pool(name="pa", bufs=1) as pa, \
                 tc.tile_pool(name="paps", bufs=1, space="PSUM") as paps:
                wq_sb = pa.tile([128, DT * 128], f32)
                nc.sync.dma_start(wq_sb[:], wqt[:])
                # imqt in quarters so projection matmuls overlap the load.
                imq_sb = pa.tile([128, DT * 512], f32)
                QT = DT // 4
                for qtr in range(4):
                    nc.sync.dma_start(
                        imq_sb[:, qtr * QT * 512:(qtr + 1) * QT * 512],
                        imqt[:, qtr * QT * 512:(qtr + 1) * QT * 512])
                imqlp_sb = pa.tile([128, DT * NLP], f32)
                nc.sync.dma_start(imqlp_sb[:], imqlp[:])
                imklp_sb = pa.tile([128, DT * NLP], f32)
                nc.sync.dma_start(imklp_sb[:], imklp[:])
                wk_sb = pa.tile([128, DT * 128], f32)
                nc.sync.dma_start(wk_sb[:], wkt[:])

                # qT[c, n] = sum_d W_q[d, c] im_q[n, d]  (all N samples)
                qt_ps = paps.tile([128, N], f32)
                for t in range(DT):
                    nc.tensor.matmul(qt_ps[:],
                                     wq_sb[:, t * 128:(t + 1) * 128],
                                     imq_sb[:, t * 512:(t + 1) * 512],
                                     start=(t == 0), stop=(t == DT - 1))
                nc.vector.tensor_copy(qtu[:], qt_ps[:])

                # l_pos shard projections (64 samples each for q and k),
                # sharing one PSUM bank in disjoint column ranges.
                aux_ps = paps.tile([128, 512], f32)
                lp_ps = aux_ps[:, 0:2 * NLP]
                for t in range(DT):
                    nc.tensor.matmul(lp_ps[:, 0:NLP],
                                     wq_sb[:, t * 128:(t + 1) * 128],
                                     imqlp_sb[:, t * NLP:(t + 1) * NLP],
                                     start=(t == 0), stop=(t == DT - 1))
                for t in range(DT):
                    nc.tensor.matmul(lp_ps[:, NLP:2 * NLP],
                                     wk_sb[:, t * 128:(t + 1) * 128],
                                     imklp_sb[:, t * NLP:(t + 1) * NLP],
                                     start=(t == 0), stop=(t == DT - 1))

                # Per-tile inverse norms: ssq_tile = (qT^2)[:, r0:r0+cnt]
                # summed over partitions via an N=1 matmul, then
                # 1/sqrt(ssq+eps) per partition. Off the critical path.
                sq_sb = pa.tile([128, N], f32)
                nc.vector.tensor_mul(sq_sb[:], qtu[:], qtu[:])
                misc_ps = aux_ps[:, 2 * NLP:2 * NLP + 64]
                inv1 = pa.tile([128, ntiles], f32)
                for ti, (r0, cnt, g) in enumerate(plan):
                    nc.tensor.matmul(misc_ps[0:cnt, ti:ti + 1],
                                     sq_sb[:, r0:r0 + cnt], ones_col[:],
                                     start=True, stop=True)
                    nc.scalar.activation(inv1[0:cnt, ti:ti + 1],
                                         misc_ps[0:cnt, ti:ti + 1],
                                         AF.Sqrt, bias=eps_biasp[0:cnt, :])
                    nc.vector.reciprocal(inv_sb[0:cnt, ti:ti + 1],
                                         inv1[0:cnt, ti:ti + 1])
                    nc.vector.tensor_scalar_mul(
                        inv_t_sb[0:cnt, ti:ti + 1],
                        inv_sb[0:cnt, ti:ti + 1], 1.0 / T)

                # l_pos shard: dot * invq_lp * invk_lp / T.
                qlp_sb = pa.tile([128, NLP], f32)
                nc.vector.tensor_copy(qlp_sb[:], lp_ps[:, 0:NLP])
                klp_sb = pa.tile([128, NLP], f32)
                nc.vector.tensor_copy(klp_sb[:], lp_ps[:, NLP:2 * NLP])
                red_sb = pa.tile([128, 3 * NLP], f32)
                nc.vector.tensor_mul(red_sb[:, 0:NLP],
                                     qlp_sb[:], qlp_sb[:])
                nc.vector.tensor_mul(red_sb[:, NLP:2 * NLP],
                                     klp_sb[:], klp_sb[:])
                nc.vector.tensor_mul(red_sb[:, 2 * NLP:3 * NLP],
                                     qlp_sb[:], klp_sb[:])
                red_ps = aux_ps[0:1, 2 * NLP + 64:5 * NLP + 64]
                nc.tensor.matmul(red_ps[:], ones_col[:], red_sb[:],
                                 start=True, stop=True)
                norml = pa.tile([1, 2 * NLP], f32)
                nc.scalar.activation(norml[:], red_ps[:, 0:2 * NLP], AF.Sqrt,
                                     bias=eps_bias[:])
                invl = pa.tile([1, 2 * NLP], f32)
                nc.vector.reciprocal(invl[:], norml[:])
                lp1 = pa.tile([1, NLP], f32)
                nc.vector.tensor_mul(lp1[:], invl[:, 0:NLP], invl[:, NLP:2 * NLP])
                lp2 = pa.tile([1, NLP], f32)
                nc.vector.tensor_mul(lp2[:], lp1[:], red_ps[:, 2 * NLP:3 * NLP])
                lp3 = pa.tile([1, NLP], f32)
                nc.vector.tensor_scalar_mul(lp3[:], lp2[:], 1.0 / T)
                nc.scalar.dma_start(lpos[:], lp3[:])

            # ---- Phase C: l_neg tiles over the queue shard ----
            # fresh0[ti]: first row of tile ti not already written by the
            # previous (overlapping) tile — only fresh rows go to HBM.
            fresh0 = []
            prev_end = 0
            for r0, cnt, g in plan:
                fresh0.append(max(r0, prev_end))
                prev_end = max(prev_end, r0 + cnt)

            with tc.tile_pool(name="qp", bufs=2) as qp, \
                 tc.tile_pool(name="sp", bufs=6) as sp, \
                 tc.tile_pool(name="cps", bufs=3, space="PSUM") as cps:
                for kc in range(NKC):
                    qch = {}
                    for g in used_groups:
                        qt_ = qp.tile([128, CW], f32, tag=f"qch{g}",
                                      name=f"qch{g}_{kc}")
                        nc.sync.dma_start(
                            qt_[:], qsh[g, :, kc * CW:(kc + 1) * CW])
                        qch[g] = qt_
                    for ti, (r0, cnt, g) in enumerate(plan):
                        f0 = fresh0[ti] - r0
                        # Two-bank PSUM tiles; each drained by a fused
                        # scale-copy (alternating DVE/ACT) into its own
                        # half-width stage slot, which goes straight out
                        # to HBM — small slots recycle fast, so out-DMA
                        # completion latency never stalls the drains.
                        for half in range(CW // 1024):
                            hw = 1024
                            ps = cps.tile([128, hw], f32, tag="ps",
                                          name=f"ps_{kc}_{ti}_{half}")
                            for sub in range(hw // 512):
                                c0 = half * hw + sub * 512
                                nc.tensor.matmul(
                                    ps[:cnt, sub * 512:(sub + 1) * 512],
                                    qtu[:, r0:r0 + cnt],
                                    qch[g][:, c0:c0 + 512],
                                    start=True, stop=True)
                            stg = sp.tile([128, hw], f32, tag="stg",
                                          name=f"stg_{kc}_{ti}_{half}")
                            if half % 2 == 0:
                                nc.vector.tensor_scalar(
                                    stg[:cnt, :], ps[:cnt, :],
                                    inv_sb[0:cnt, ti:ti + 1], 1.0 / T,
                                    op0=mybir.AluOpType.mult,
                                    op1=mybir.AluOpType.mult)
                            else:
                                nc.scalar.activation(
                                    stg[:cnt, :], ps[:cnt, :], AF.Copy,
                                    scale=inv_t_sb[0:cnt, ti:ti + 1])
                            nc.scalar.dma_start(
                                lneg[fresh0[ti]:r0 + cnt,
                                     kc * CW + half * hw:
                                     kc * CW + (half + 1) * hw],
                                stg[f0:cnt, :])

    nc.compile()
    return nc


def _get_program(plan):
    if plan not in _prog_cache:
        _prog_cache[plan] = _build(plan)
    return _prog_cache[plan]


def _tile_cols(x, ncols):
    """[n, D] -> [128, DT*n] with [p, t*n+j] = x[j, t*128+p]."""
    n = x.shape[0]
    assert n == ncols
    return np.ascontiguousarray(
        x.T.reshape(DT, 128, n).transpose(1, 0, 2).reshape(128, DT * n))


def _stage_inputs(im_q, im_k, W_q, W_k, queues, label):
    f32 = np.float32
    im_q = np.asarray(im_q, dtype=f32)
    im_k = np.asarray(im_k, dtype=f32)
    W_q = np.asarray(W_q, dtype=f32)
    W_k = np.asarray(W_k, dtype=f32)
    queues = np.asarray(queues, dtype=f32)
    label = np.asarray(label)

    route = ((label.astype(np.int64) - 1) % G).astype(np.int64)
    order = np.argsort(route, kind="stable")
    counts = np.bincount(route, minlength=G)
    plan = _plan_from_counts(counts)

    im_q_s = im_q[order]
    im_k_s = im_k[order]

    imqt = _tile_cols(im_q_s, N)
    wqt = np.ascontiguousarray(
        W_q.reshape(DT, 128, C).transpose(1, 0, 2).reshape(128, DT * C))
    wkt = np.ascontiguousarray(
        W_k.reshape(DT, 128, C).transpose(1, 0, 2).reshape(128, DT * C))

    in_maps = []
    for i in range(NCORES):
        sl = slice(i * NLP, (i + 1) * NLP)
        in_maps.append({
            "imqt": imqt, "wqt": wqt, "wkt": wkt,
            "imqlp": _tile_cols(im_q_s[sl], NLP),
            "imklp": _tile_cols(im_k_s[sl], NLP),
            "qsh": np.ascontiguousarray(queues[:, :, i * KSH:(i + 1) * KSH]),
        })
    return plan, order, in_maps


def kernel(im_q, im_k, W_q, W_k, queues, label):
    from concourse.bass_utils import run_bass_kernel_spmd

    plan, order, in_maps = _stage_inputs(im_q, im_k, W_q, W_k, queues, label)
    nc = _get_program(plan)
    res = run_bass_kernel_spmd(nc, in_maps, core_ids=list(range(NCORES)))

    logits = np.empty((N, 1 + K), dtype=np.float32)
    lpos_sorted = np.concatenate(
        [res.results[i]["lpos"][0] for i in range(NCORES)])
    logits[order, 0] = lpos_sorted
    for i in range(NCORES):
        logits[order, 1 + i * KSH:1 + (i + 1) * KSH] = res.results[i]["lneg"]
    labels = np.zeros(N, dtype=np.int32)
    return logits, labels
